# revision 6
# baseline (speedup 1.0000x reference)
"""Trainium2 Bass kernel for quantized int8 linear (per-token absmax activation
quantization + int8 weight GEMM + dequant), tensor-parallel over 8 NeuronCores.

Reference computation (see problem):
    mag  = max|x| per token row                      [B,S,1]
    a_s  = mag / 127
    a    = round(x / a_s)  (clipped to int8, but |x/a_s| <= 127 by construction)
    acc  = a @ W.T   (int8 x int8 -> int32)
    out  = acc * a_s * weight_scale.T                [B,S,OUT]

Strategy: data-parallel over tokens (8192 tokens -> 1024 per core). Each core
gets its token shard of x, the full weight (pre-transposed to [IN,OUT] and cast
to bf16 on host - int8 values are exact in bf16), and the weight_scale
broadcast to [128, OUT]. The matmul runs in bf16 with fp32 PSUM accumulation,
which is numerically exact here (products are integers <= 16129, partial sums
stay far below 2^24).

Per-core pipeline:
  Phase A (quantize): per 128-token block: DMA x tile, DVE absmax-reduce,
    inv = 127/mag (exact reciprocal), ACT computes x*inv + 1.5*2^23 (the fp32
    magic-number round-to-nearest-even), DVE subtracts the magic constant and
    casts to bf16, DMA the quantized block to a DRAM scratch.
  Phase B (matmul): 32 DMA-transposes load a^T as [128(K), 1024(tokens)]
    slices; weight panels [128, 32, 512] bf16 stream from DRAM; 32-deep K
    accumulation into PSUM per (token-block, N-panel); dequant = ACT scale by
    per-token a_s (per-partition AP scale) then DVE multiply by the
    weight_scale broadcast tile; DMA out.
"""

import os
from contextlib import ExitStack

import numpy as np
import ml_dtypes

import concourse.bass as bass
import concourse.mybir as mybir
import concourse.tile as tile
from concourse.bass_utils import run_bass_kernel_spmd

F32 = mybir.dt.float32
BF16 = mybir.dt.bfloat16

N_CORES = 8
B, S, IN, OUT = 4, 2048, 4096, 4096
M_TOTAL = B * S
M_CORE = M_TOTAL // N_CORES

# 1.5 * 2**23: adding then subtracting rounds fp32 to nearest-even integer
# for |v| <= 2**22.
C_ROUND = 12582912.0

# Stash of the most recent BassKernelResults (exec_time_ns etc.) for test.py.
LAST_RESULTS = None


def _split_waits(nc, max_attached=1):
    """Walrus codegen in this toolchain fits only one sync-wait per engine
    instruction struct ("Too many sync wait commands" otherwise). Move extra
    waits onto same-engine InstNoOps inserted immediately before the
    instruction — identical blocking semantics, acceptable encoding."""
    for f in nc.m.functions:
        for b in f.blocks:
            new = []
            for inst in b.instructions:
                si = inst.sync_info
                if si is not None and si.on_wait and len(si.on_wait) > max_attached:
                    waits = list(si.on_wait)
                    for w in waits[max_attached:]:
                        new.append(mybir.InstNoOp(
                            name=nc.get_next_instruction_name(),
                            sync_info=mybir.SyncInfo(on_wait=[w], on_update=[]),
                            bass_nofuse=True,
                            engine=inst.engine,
                            ins=[], outs=[],
                        ))
                    inst.sync_info = mybir.SyncInfo(
                        on_wait=waits[:max_attached],
                        on_update=list(si.on_update),
                    )
                new.append(inst)
            b.instructions[:] = new


def build_kernel(M=M_CORE, K=IN, N=OUT, NP=512, psum_bufs=4):
    """Build the per-core Bass program. M tokens, K contraction, N out
    features, NP = N-panel width per PSUM tile."""
    assert M % 128 == 0 and K % 128 == 0 and N % NP == 0 and NP <= 512
    MB = M // 128   # token blocks
    KB = K // 128   # contraction blocks
    NPAN = N // NP  # N panels

    nc = bass.Bass()
    x_h = nc.declare_dram_parameter("x", [M, K], F32, isOutput=False)
    wt_h = nc.declare_dram_parameter("wt", [K, N], BF16, isOutput=False)
    wsb_h = nc.declare_dram_parameter("wsb", [128, N], F32, isOutput=False)
    out_h = nc.declare_dram_parameter("out", [M, N], F32, isOutput=True)

    with ExitStack() as ctx:
        tc = ctx.enter_context(tile.TileContext(nc))
        dram = ctx.enter_context(tc.tile_pool(name="dram", bufs=1, space="DRAM"))
        xpool = ctx.enter_context(tc.tile_pool(name="x", bufs=2))
        apool = ctx.enter_context(tc.tile_pool(name="a", bufs=2))
        stats = ctx.enter_context(tc.tile_pool(name="stats", bufs=2))
        persist = ctx.enter_context(tc.tile_pool(name="persist", bufs=1))
        wpool = ctx.enter_context(tc.tile_pool(name="w", bufs=2))
        wspool = ctx.enter_context(tc.tile_pool(name="ws", bufs=2))
        dqpool = ctx.enter_context(tc.tile_pool(name="dq", bufs=3))
        psum = ctx.enter_context(tc.tile_pool(name="psum", bufs=psum_bufs, space="PSUM"))

        a_dram = dram.tile([M, K], BF16)
        a_s = persist.tile([128, MB], F32)      # per-token dequant scale, col m
        aT = persist.tile([128, KB, M], BF16)   # transposed quantized acts
        c_tile = persist.tile([128, 1], F32)    # round-magic constant as AP
        nc.vector.memset(c_tile[:], C_ROUND)

        # ---- Phase A: per-token absmax quantization ----
        for m in range(MB):
            xt = xpool.tile([128, K], F32)
            nc.sync.dma_start(xt[:], x_h[m * 128:(m + 1) * 128, :])

            mag = stats.tile([128, 1], F32, tag="mag")
            nc.vector.tensor_reduce(
                mag[:], xt[:], axis=mybir.AxisListType.X,
                op=mybir.AluOpType.max, apply_absolute_value=True,
            )
            # guard mag==0 (all-zero row): clamp before reciprocal
            nc.vector.tensor_scalar_max(mag[:], mag[:], 1e-30)
            inv = stats.tile([128, 1], F32, tag="inv")
            nc.vector.reciprocal(inv[:], mag[:])                  # exact 1/mag
            nc.vector.tensor_scalar_mul(inv[:], inv[:], 127.0)    # 127/mag
            nc.vector.tensor_scalar_mul(a_s[:, m:m + 1], mag[:], 1.0 / 127.0)

            # q = x * inv + C (ACT, per-partition scale/bias APs), then
            # a = q - C cast to bf16 (ACT Copy, immediate bias)
            # == round-half-even(x*inv). (DVE tensor_scalar is avoided for
            # these: TensorScalarPtr's encoding fits only one sync wait.)
            nc.scalar.activation(
                xt[:], xt[:], mybir.ActivationFunctionType.Identity,
                bias=c_tile[:], scale=inv[:],
            )
            at = apool.tile([128, K], BF16)
            nc.scalar.activation(
                at[:], xt[:], mybir.ActivationFunctionType.Copy, bias=-C_ROUND
            )
            nc.sync.dma_start(a_dram[m * 128:(m + 1) * 128, :], at[:])

        # ---- Phase B: transposed activation loads + GEMM + dequant ----
        for k in range(KB):
            nc.sync.dma_start_transpose(
                aT[:, k, :], a_dram[:, k * 128:(k + 1) * 128]
            )

        wt_view = wt_h.rearrange("(ko p) n -> p ko n", p=128)
        for n in range(NPAN):
            wt = wpool.tile([128, KB, NP], BF16)
            nc.sync.dma_start(wt[:], wt_view[:, :, n * NP:(n + 1) * NP])
            wsb_t = wspool.tile([128, NP], F32)
            nc.sync.dma_start(wsb_t[:], wsb_h[:, n * NP:(n + 1) * NP])

            for m in range(MB):
                ps = psum.tile([128, NP], F32)
                for k in range(KB):
                    nc.tensor.matmul(
                        ps[:],
                        lhsT=aT[:, k, m * 128:(m + 1) * 128],
                        rhs=wt[:, k, :],
                        start=(k == 0),
                        stop=(k == KB - 1),
                    )
                dq = dqpool.tile([128, NP], F32)
                nc.scalar.activation(
                    dq[:], ps[:], mybir.ActivationFunctionType.Copy,
                    scale=a_s[:, m:m + 1],
                )
                nc.vector.tensor_tensor(
                    dq[:], dq[:], wsb_t[:], mybir.AluOpType.mult
                )
                nc.sync.dma_start(
                    out_h[m * 128:(m + 1) * 128, n * NP:(n + 1) * NP], dq[:]
                )

    _split_waits(nc)
    return nc


def _shard_inputs(x, weight, weight_scale):
    """Host-side shard prep: token-shard x, pre-transpose+cast weight."""
    x2d = np.asarray(x, dtype=np.float32).reshape(M_TOTAL, IN)
    wt = np.ascontiguousarray(
        np.asarray(weight).T.astype(ml_dtypes.bfloat16)
    )  # [IN, OUT] bf16; int8 values are exact in bf16
    wsb = np.ascontiguousarray(
        np.broadcast_to(
            np.asarray(weight_scale, dtype=np.float32).reshape(1, OUT), (128, OUT)
        )
    )
    in_maps = []
    for i in range(N_CORES):
        in_maps.append({
            "x": np.ascontiguousarray(x2d[i * M_CORE:(i + 1) * M_CORE]),
            "wt": wt,
            "wsb": wsb,
        })
    return in_maps


_NC_CACHE = None


def kernel(x, weight, weight_scale):
    global LAST_RESULTS, _NC_CACHE
    if _NC_CACHE is None:
        _NC_CACHE = build_kernel()
    nc = _NC_CACHE
    in_maps = _shard_inputs(x, weight, weight_scale)
    trace = bool(int(os.environ.get("KERNEL_TRACE", "0")))
    res = run_bass_kernel_spmd(nc, in_maps, list(range(N_CORES)), trace=trace)
    LAST_RESULTS = res
    out = np.concatenate(
        [res.results[i]["out"] for i in range(N_CORES)], axis=0
    )
    return out.reshape(B, S, OUT).astype(np.float32)
